# revision 11
# baseline (speedup 1.0000x reference)
"""BitNetLinear Trainium2 kernel (8 NeuronCores, SPMD data-parallel).

y = round(clip(x, +-127*s)/s)*s @ (ternary(W))^T + ternary(b)
with s = exp2(floor(log2(max|x|/127 + eps))) a power of two (global over x).

Sharding: batch dim (8) -> one batch of [4096, 1024] per core.
Host prep: x shard transposed to [in, rows] (PE contracts over partitions);
weight/bias ternary-quantized on host (reference does this in __init__);
ternary weight shipped as bf16 [in, out].

Device: phase 1 streams x computing local absmax -> partition_all_reduce ->
512B AllReduce(max) across the 8 cores; scale = exponent-masked (exact
power of two); x quantized to integer-valued bf16 (round-half-even via
+-1.5*2^23 trick); bf16 matmul with fp32 PSUM accumulation is exact integer
arithmetic (|x_int| <= 127, w in {-1,0,1}, |acc| < 2^24); result scaled by
s*gamma and bias added.
"""

import numpy as np
import ml_dtypes
from contextlib import ExitStack

import concourse.bass as bass
import concourse.mybir as mybir
import concourse.tile as tile
from concourse import bacc, bass_isa, bass_utils

F32 = mybir.dt.float32
BF16 = mybir.dt.bfloat16
I32 = mybir.dt.int32

N_CORES = 8
P = 128
IN_F = 1024
OUT_F = 1024
KC = IN_F // P          # 8 contraction chunks
RSUB = 256              # rows loaded/quantized per chunk
ROUND_C = 12582912.0    # 1.5 * 2**23: float32 round-half-even trick
EPS = 1e-8
QMAX = 127.0


def build_program(rows: int = 4096, num_cores: int = N_CORES) -> bacc.Bacc:
    assert rows % RSUB == 0
    nc = bacc.Bacc(
        "TRN2",
        target_bir_lowering=False,
        debug=False,
        enable_asserts=False,
        num_devices=num_cores,
    )
    xt = nc.dram_tensor("xt", (IN_F, rows), F32, kind="ExternalInput").ap()
    wq = nc.dram_tensor("wq", (IN_F, OUT_F), BF16, kind="ExternalInput").ap()
    bq = nc.dram_tensor("bq", (1, OUT_F), F32, kind="ExternalInput").ap()
    gq = nc.dram_tensor("gq", (1, 1), F32, kind="ExternalInput").ap()
    y = nc.dram_tensor("y", (rows, OUT_F), F32, kind="ExternalOutput").ap()
    # Collectives cannot target I/O tensors; bounce through internal DRAM.
    cc_in1 = nc.dram_tensor("cc_in1", (P, 1), F32).ap()
    cc_out1 = nc.dram_tensor("cc_out1", (P, 1), F32).ap()
    cc_in2 = nc.dram_tensor("cc_in2", (P, 1), F32).ap()
    cc_out2 = nc.dram_tensor("cc_out2", (P, 1), F32).ap()

    with tile.TileContext(nc, num_cores=num_cores) as tc, ExitStack() as ctx:
        consts = ctx.enter_context(tc.tile_pool(name="consts", bufs=1))

        # --- phase 1: local absmax of the x shard, split in two halves with
        # a pipelined AllReduce per half. AR1 launches mid-read and absorbs
        # the cross-core launch skew + ncfw wake latency while the second
        # half of x still streams; AR2 then sees near-aligned peers.
        half = rows // 2
        xt_chunks = xt.rearrange("(c p) (h r) -> c h p r", p=P, h=2)
        rg = [list(range(num_cores))]
        gmax_parts = []
        with tc.tile_pool(name="xmax", bufs=4) as xpool:
            for stage, (cc_in, cc_out) in enumerate(
                [(cc_in1, cc_out1), (cc_in2, cc_out2)]
            ):
                partials = consts.tile([P, KC], F32, tag=f"partials{stage}")
                for jj in range(KC):
                    j = stage * KC + jj
                    xsb = xpool.tile([P, half], F32)
                    nc.sync.dma_start(out=xsb, in_=xt_chunks[j // 2, j % 2])
                    nc.vector.tensor_reduce(
                        out=partials[:, jj : jj + 1],
                        in_=xsb,
                        axis=mybir.AxisListType.X,
                        op=mybir.AluOpType.max,
                        apply_absolute_value=True,
                    )
                lmax = consts.tile([P, 1], F32, tag=f"lmax{stage}")
                nc.vector.tensor_reduce(
                    out=lmax,
                    in_=partials,
                    axis=mybir.AxisListType.X,
                    op=mybir.AluOpType.max,
                )
                gmax_l = consts.tile([P, 1], F32, tag=f"gmax_l{stage}")
                nc.gpsimd.partition_all_reduce(
                    gmax_l, lmax, channels=P, reduce_op=bass_isa.ReduceOp.max
                )
                nc.sync.dma_start(out=cc_in, in_=gmax_l)
                nc.gpsimd.collective_compute(
                    "AllReduce",
                    mybir.AluOpType.max,
                    replica_groups=rg,
                    ins=[cc_in.opt()],
                    outs=[cc_out.opt()],
                )
                gmax_p = consts.tile([P, 1], F32, tag=f"gmax_p{stage}")
                nc.sync.dma_start(out=gmax_p, in_=cc_out)
                gmax_parts.append(gmax_p)

        # --- constants: ternary weight [p, kc, o], bias row, gamma scalar ---
        w_sb = consts.tile([P, KC, OUT_F], BF16)
        nc.sync.dma_start(out=w_sb, in_=wq.rearrange("(c p) o -> p c o", p=P))
        bias_sb = consts.tile([P, OUT_F], F32)
        nc.sync.dma_start(out=bias_sb, in_=bq.to_broadcast((P, OUT_F)))
        gamma_sb = consts.tile([P, 1], F32)
        nc.sync.dma_start(out=gamma_sb, in_=gq.to_broadcast((P, 1)))

        gmax = consts.tile([P, 1], F32)
        nc.vector.tensor_tensor(
            out=gmax,
            in0=gmax_parts[0],
            in1=gmax_parts[1],
            op=mybir.AluOpType.max,
        )

        # --- PE warmup: junk matmuls gated on the AllReduce result. They
        # fill the post-collective bubble while the scale chain + first
        # quantize run, flipping HAM to full clock before the real matmuls.
        warm_rhs = consts.tile([P, 512], BF16)
        nc.vector.memset(warm_rhs, 0.0)
        nc.vector.tensor_copy(out=warm_rhs[:, 0:1], in_=gmax)
        with tc.tile_pool(name="warm_ps", bufs=1, space="PSUM") as warm_pool:
            warm_ps = warm_pool.tile([P, 512], F32)
            for _ in range(16):
                nc.tensor.matmul(
                    warm_ps,
                    lhsT=w_sb[:, 0, 0:P],
                    rhs=warm_rhs,
                    start=True,
                    stop=True,
                )

        # --- scale: s = exp2(floor(log2(m/127 + eps))) via exponent masking ---
        v_t = consts.tile([P, 1], F32)
        nc.vector.tensor_scalar(
            out=v_t,
            in0=gmax,
            scalar1=float(np.float32(1.0 / 127.0)),
            scalar2=float(np.float32(EPS)),
            op0=mybir.AluOpType.mult,
            op1=mybir.AluOpType.add,
        )
        mask_t = consts.tile([P, 1], I32)
        nc.vector.memset(mask_t, -8388608)  # 0xFF800000: sign+exponent mask
        expc_t = consts.tile([P, 1], I32)
        nc.vector.memset(expc_t, 0x7F000000)  # bits of (254<<23)
        s_t = consts.tile([P, 1], F32)
        nc.vector.tensor_tensor(
            out=s_t.bitcast(I32),
            in0=v_t.bitcast(I32),
            in1=mask_t,
            op=mybir.AluOpType.bitwise_and,
        )
        # 1/s for a power of two: exponent bits of (254<<23) - s_bits
        inv_t = consts.tile([P, 1], F32)
        nc.vector.tensor_tensor(
            out=inv_t.bitcast(I32),
            in0=expc_t,
            in1=s_t.bitcast(I32),
            op=mybir.AluOpType.subtract,
        )
        c_t = consts.tile([P, 1], F32)  # s * gamma_w
        nc.vector.tensor_mul(out=c_t, in0=s_t, in1=gamma_sb)
        negc_t = consts.tile([P, 1], F32)
        nc.vector.memset(negc_t, -ROUND_C)

        # --- phase 2: quantize + matmul + scale/bias + store ---
        # per 256-row chunk: [P(in), KC, RSUB] tiles of x^T
        xt_cols = xt.rearrange("(c p) (t r) -> t p c r", p=P, r=RSUB)
        y_rows = y.rearrange("(t p) o -> t p o", p=P)
        nhalf = OUT_F // 512
        with (
            tc.tile_pool(name="xq", bufs=6) as xq_pool,
            tc.tile_pool(name="tq", bufs=3) as tq_pool,
            tc.tile_pool(name="ub", bufs=3) as ub_pool,
            tc.tile_pool(name="xi", bufs=4) as xi_pool,
            tc.tile_pool(name="yo", bufs=4) as yo_pool,
            tc.tile_pool(name="ps", bufs=4, space="PSUM") as ps_pool,
        ):
            for t in range(rows // RSUB):
                xc = xq_pool.tile([P, KC, RSUB], F32)
                nc.sync.dma_start(out=xc, in_=xt_cols[t])
                # t = x/s + C  (mult is exact: s a power of two)
                tq = tq_pool.tile([P, KC, RSUB], F32)
                nc.vector.tensor_scalar(
                    out=tq,
                    in0=xc,
                    scalar1=inv_t,
                    scalar2=ROUND_C,
                    op0=mybir.AluOpType.mult,
                    op1=mybir.AluOpType.add,
                )
                # u = t - C  -> integer-valued, cast to bf16 (exact, |u|<256)
                ub = ub_pool.tile([P, KC, RSUB], BF16)
                nc.scalar.activation(
                    out=ub,
                    in_=tq,
                    func=mybir.ActivationFunctionType.Identity,
                    bias=negc_t,
                    scale=1.0,
                )
                # x_int = clip(u, -127, 127)
                xi = xi_pool.tile([P, KC, RSUB], BF16)
                nc.vector.tensor_scalar(
                    out=xi,
                    in0=ub,
                    scalar1=-127.0,
                    scalar2=127.0,
                    op0=mybir.AluOpType.max,
                    op1=mybir.AluOpType.min,
                )
                for h in range(RSUB // P):
                    ps = ps_pool.tile([P, OUT_F], F32)
                    for k in range(KC):
                        for n in range(nhalf):
                            nc.tensor.matmul(
                                ps[:, n * 512 : (n + 1) * 512],
                                lhsT=xi[:, k, h * P : (h + 1) * P],
                                rhs=w_sb[:, k, n * 512 : (n + 1) * 512],
                                start=(k == 0),
                                stop=(k == KC - 1),
                            )
                    yo = yo_pool.tile([P, OUT_F], F32)
                    nc.scalar.activation(
                        out=yo,
                        in_=ps,
                        func=mybir.ActivationFunctionType.Copy,
                        bias=0.0,
                        scale=c_t,
                    )
                    nc.vector.tensor_add(out=yo, in0=yo, in1=bias_sb)
                    nc.sync.dma_start(
                        out=y_rows[t * (RSUB // P) + h], in_=yo
                    )

    nc.compile()
    return nc


def quantize_params(weight: np.ndarray, bias: np.ndarray):
    """Ternary-quantize weight/bias exactly as the reference (f64 math whose
    f32 rounding matches jax-f32; verified margins are orders of magnitude
    above f32 accumulation differences)."""
    w64 = weight.astype(np.float64)
    g_w = np.float32(np.abs(w64).mean())
    wi = np.clip(np.round(w64 / (np.float64(g_w) + EPS)), -1.0, 1.0)
    b64 = bias.astype(np.float64)
    g_b = np.float32(np.abs(b64).mean())
    bi = np.clip(np.round(b64 / (np.float64(g_b) + EPS)), -1.0, 1.0)
    bq = (bi * np.float64(g_b)).astype(np.float32)  # exact: {-g_b, 0, g_b}
    return wi, g_w, bq


_PROGRAM_CACHE: dict[int, bacc.Bacc] = {}


def _get_program(rows: int) -> bacc.Bacc:
    if rows not in _PROGRAM_CACHE:
        _PROGRAM_CACHE[rows] = build_program(rows)
    return _PROGRAM_CACHE[rows]


def prepare_in_maps(x: np.ndarray, weight: np.ndarray, bias: np.ndarray):
    x = np.asarray(x, dtype=np.float32)
    weight = np.asarray(weight, dtype=np.float32)
    bias = np.asarray(bias, dtype=np.float32)
    batch, rows, in_f = x.shape
    assert batch == N_CORES and in_f == IN_F and weight.shape == (OUT_F, IN_F)

    wi, g_w, bq = quantize_params(weight, bias)
    wq_t = np.ascontiguousarray(wi.T).astype(ml_dtypes.bfloat16)  # [in, out]
    bq_row = np.ascontiguousarray(bq.reshape(1, OUT_F))
    gq = np.array([[g_w]], dtype=np.float32)

    in_maps = []
    for c in range(N_CORES):
        in_maps.append(
            {
                "xt": np.ascontiguousarray(x[c].T),
                "wq": wq_t,
                "bq": bq_row,
                "gq": gq,
            }
        )
    return in_maps, rows


def kernel(x: np.ndarray, weight: np.ndarray, bias: np.ndarray) -> np.ndarray:
    in_maps, rows = prepare_in_maps(x, weight, bias)
    nc = _get_program(rows)
    res = bass_utils.run_bass_kernel_spmd(nc, in_maps, core_ids=list(range(N_CORES)))
    return np.stack([res.results[c]["y"] for c in range(N_CORES)], axis=0)


# revision 12
# speedup vs baseline: 1.1441x; 1.1441x over previous
"""BitNetLinear Trainium2 kernel (8 NeuronCores, SPMD data-parallel).

y = round(clip(x, +-127*s)/s)*s @ (ternary(W))^T + ternary(b)
with s = exp2(floor(log2(max|x|/127 + eps))) a power of two (global over x).

Sharding: batch dim (8) -> one batch of [4096, 1024] per core.
Host prep: x shard transposed to [in, rows] (PE contracts over partitions);
weight/bias ternary-quantized on host (reference does this in __init__);
ternary weight shipped as bf16 [in, out].

Device: phase 1 streams x computing the local absmax. Because
floor(log2(.)) commutes with max, the global power-of-two scale equals the
max of the per-core local scales, so each core SPECULATES with its local
scale and starts quantize+matmul immediately while a 512B AllReduce(max)
flies concurrently on the collectives hardware. At the end each core
compares its speculative scale against the global one and, on mismatch,
re-runs the (exact) quantize+matmul with the global scale under a
conditional branch. x is quantized to integer-valued bf16 (round-half-even
via the +-1.5*2^23 trick); the bf16 matmul with fp32 PSUM accumulation is
exact integer arithmetic (|x_int| <= 127, w in {-1,0,1}, |acc| < 2^24);
the result is scaled by s*gamma_w and the ternary bias is added.
"""

import numpy as np
import ml_dtypes
from contextlib import ExitStack

import concourse.bass as bass
import concourse.mybir as mybir
import concourse.tile as tile
from concourse import bacc, bass_isa, bass_utils

F32 = mybir.dt.float32
BF16 = mybir.dt.bfloat16
I32 = mybir.dt.int32

N_CORES = 8
P = 128
IN_F = 1024
OUT_F = 1024
KC = IN_F // P          # 8 contraction chunks
RSUB = 256              # rows quantized per chunk
ROUND_C = 12582912.0    # 1.5 * 2**23: float32 round-half-even trick
EPS = 1e-8


def _emit_scale_chain(nc, consts, gmax, gamma_sb, mask_t, expc_t, tag):
    """From a [P,1] absmax tile, compute s = exp2(floor(log2(m/127+eps)))
    via exponent masking, 1/s via exponent arithmetic, and c = s*gamma."""
    v_t = consts.tile([P, 1], F32, tag=f"v_{tag}")
    nc.vector.tensor_scalar(
        out=v_t,
        in0=gmax,
        scalar1=float(np.float32(1.0 / 127.0)),
        scalar2=float(np.float32(EPS)),
        op0=mybir.AluOpType.mult,
        op1=mybir.AluOpType.add,
    )
    s_t = consts.tile([P, 1], F32, tag=f"s_{tag}")
    nc.vector.tensor_tensor(
        out=s_t.bitcast(I32),
        in0=v_t.bitcast(I32),
        in1=mask_t,
        op=mybir.AluOpType.bitwise_and,
    )
    inv_t = consts.tile([P, 1], F32, tag=f"inv_{tag}")
    nc.vector.tensor_tensor(
        out=inv_t.bitcast(I32),
        in0=expc_t,
        in1=s_t.bitcast(I32),
        op=mybir.AluOpType.subtract,
    )
    c_t = consts.tile([P, 1], F32, tag=f"c_{tag}")
    nc.vector.tensor_mul(out=c_t, in0=s_t, in1=gamma_sb)
    return s_t, inv_t, c_t


def _emit_phase2(nc, pools, rows, xt_cols, y_rows, w_sb, bias_sb, negc_t,
                 inv_t, c_t):
    """Quantize x with 1/s, matmul against the ternary weight, scale by c,
    add bias, store y."""
    xq_pool, tq_pool, ub_pool, xi_pool, yo_pool, ps_pool = pools
    nhalf = OUT_F // 512
    for t in range(rows // RSUB):
        xc = xq_pool.tile([P, KC, RSUB], F32, tag="xc")
        nc.sync.dma_start(out=xc, in_=xt_cols[t])
        tq = tq_pool.tile([P, KC, RSUB], F32, tag="tq")
        nc.vector.tensor_scalar(
            out=tq,
            in0=xc,
            scalar1=inv_t,
            scalar2=ROUND_C,
            op0=mybir.AluOpType.mult,
            op1=mybir.AluOpType.add,
        )
        ub = ub_pool.tile([P, KC, RSUB], BF16, tag="ub")
        nc.scalar.activation(
            out=ub,
            in_=tq,
            func=mybir.ActivationFunctionType.Identity,
            bias=negc_t,
            scale=1.0,
        )
        xi = xi_pool.tile([P, KC, RSUB], BF16, tag="xi")
        nc.vector.tensor_scalar(
            out=xi,
            in0=ub,
            scalar1=-127.0,
            scalar2=127.0,
            op0=mybir.AluOpType.max,
            op1=mybir.AluOpType.min,
        )
        for h in range(RSUB // P):
            ps = ps_pool.tile([P, OUT_F], F32, tag="ps")
            for k in range(KC):
                for n in range(nhalf):
                    nc.tensor.matmul(
                        ps[:, n * 512 : (n + 1) * 512],
                        lhsT=xi[:, k, h * P : (h + 1) * P],
                        rhs=w_sb[:, k, n * 512 : (n + 1) * 512],
                        start=(k == 0),
                        stop=(k == KC - 1),
                    )
            yo = yo_pool.tile([P, OUT_F], F32, tag="yo")
            nc.scalar.activation(
                out=yo,
                in_=ps,
                func=mybir.ActivationFunctionType.Copy,
                bias=0.0,
                scale=c_t,
            )
            nc.vector.tensor_add(out=yo, in0=yo, in1=bias_sb)
            nc.sync.dma_start(out=y_rows[t * (RSUB // P) + h], in_=yo)


def build_program(rows: int = 4096, num_cores: int = N_CORES,
                  speculate: bool = True) -> bacc.Bacc:
    assert rows % RSUB == 0
    nc = bacc.Bacc(
        "TRN2",
        target_bir_lowering=False,
        debug=False,
        enable_asserts=False,
        num_devices=num_cores,
    )
    xt = nc.dram_tensor("xt", (IN_F, rows), F32, kind="ExternalInput").ap()
    wq = nc.dram_tensor("wq", (IN_F, OUT_F), BF16, kind="ExternalInput").ap()
    bq = nc.dram_tensor("bq", (1, OUT_F), F32, kind="ExternalInput").ap()
    gq = nc.dram_tensor("gq", (1, 1), F32, kind="ExternalInput").ap()
    y = nc.dram_tensor("y", (rows, OUT_F), F32, kind="ExternalOutput").ap()
    # Collectives cannot target I/O tensors; bounce through internal DRAM.
    cc_in = nc.dram_tensor("cc_in", (P, 1), F32).ap()
    cc_out = nc.dram_tensor("cc_out", (P, 1), F32).ap()

    with tile.TileContext(nc, num_cores=num_cores) as tc, ExitStack() as ctx:
        consts = ctx.enter_context(tc.tile_pool(name="consts", bufs=1))

        mask_t = consts.tile([P, 1], I32)
        nc.vector.memset(mask_t, -8388608)  # 0xFF800000: sign+exponent mask
        expc_t = consts.tile([P, 1], I32)
        nc.vector.memset(expc_t, 0x7F000000)  # bits of (254<<23)
        negc_t = consts.tile([P, 1], F32)
        nc.vector.memset(negc_t, -ROUND_C)

        # --- phase 1: local absmax of the x shard ---
        half = rows // 2
        xt_chunks = xt.rearrange("(c p) (h r) -> c h p r", p=P, h=2)
        partials = consts.tile([P, 2 * KC], F32)
        with tc.tile_pool(name="xmax", bufs=4) as xpool:
            for j in range(2 * KC):
                xsb = xpool.tile([P, half], F32)
                nc.sync.dma_start(out=xsb, in_=xt_chunks[j // 2, j % 2])
                nc.vector.tensor_reduce(
                    out=partials[:, j : j + 1],
                    in_=xsb,
                    axis=mybir.AxisListType.X,
                    op=mybir.AluOpType.max,
                    apply_absolute_value=True,
                )
        lmax = consts.tile([P, 1], F32)
        nc.vector.tensor_reduce(
            out=lmax, in_=partials, axis=mybir.AxisListType.X,
            op=mybir.AluOpType.max,
        )
        gmax_l = consts.tile([P, 1], F32)
        nc.gpsimd.partition_all_reduce(
            gmax_l, lmax, channels=P, reduce_op=bass_isa.ReduceOp.max
        )

        # --- global max across the 8 cores (gpsimd queue so the bounce
        # DMAs never block the sync queue that feeds phase 2) ---
        nc.gpsimd.dma_start(out=cc_in, in_=gmax_l)
        nc.gpsimd.collective_compute(
            "AllReduce",
            mybir.AluOpType.max,
            replica_groups=[list(range(num_cores))],
            ins=[cc_in.opt()],
            outs=[cc_out.opt()],
        )
        gmax_g = consts.tile([P, 1], F32)
        nc.gpsimd.dma_start(out=gmax_g, in_=cc_out)

        # --- constants: ternary weight [p, kc, o], bias row, gamma scalar ---
        w_sb = consts.tile([P, KC, OUT_F], BF16)
        nc.sync.dma_start(out=w_sb, in_=wq.rearrange("(c p) o -> p c o", p=P))
        bias_sb = consts.tile([P, OUT_F], F32)
        nc.sync.dma_start(out=bias_sb, in_=bq.to_broadcast((P, OUT_F)))
        gamma_sb = consts.tile([P, 1], F32)
        nc.sync.dma_start(out=gamma_sb, in_=gq.to_broadcast((P, 1)))

        # --- PE warmup: junk matmuls gated on the local max; they flip HAM
        # to full clock while the scale chain + first quantize run ---
        warm_rhs = consts.tile([P, 512], BF16)
        nc.vector.memset(warm_rhs, 0.0)
        nc.vector.tensor_copy(out=warm_rhs[:, 0:1], in_=gmax_l)
        with tc.tile_pool(name="warm_ps", bufs=1, space="PSUM") as warm_pool:
            warm_ps = warm_pool.tile([P, 512], F32)
            for _ in range(16):
                nc.tensor.matmul(
                    warm_ps, lhsT=w_sb[:, 0, 0:P], rhs=warm_rhs,
                    start=True, stop=True,
                )

        xt_cols = xt.rearrange("(c p) (t r) -> t p c r", p=P, r=RSUB)
        y_rows = y.rearrange("(t p) o -> t p o", p=P)

        with (
            tc.tile_pool(name="xq", bufs=6) as xq_pool,
            tc.tile_pool(name="tq", bufs=3) as tq_pool,
            tc.tile_pool(name="ub", bufs=3) as ub_pool,
            tc.tile_pool(name="xi", bufs=4) as xi_pool,
            tc.tile_pool(name="yo", bufs=4) as yo_pool,
            tc.tile_pool(name="ps", bufs=4, space="PSUM") as ps_pool,
        ):
            pools = (xq_pool, tq_pool, ub_pool, xi_pool, yo_pool, ps_pool)
            if not speculate:
                _, inv_g, c_g = _emit_scale_chain(
                    nc, consts, gmax_g, gamma_sb, mask_t, expc_t, "g")
                _emit_phase2(nc, pools, rows, xt_cols, y_rows, w_sb, bias_sb,
                             negc_t, inv_g, c_g)
            else:
                s_l, inv_l, c_l = _emit_scale_chain(
                    nc, consts, gmax_l, gamma_sb, mask_t, expc_t, "l")
                _emit_phase2(nc, pools, rows, xt_cols, y_rows, w_sb, bias_sb,
                             negc_t, inv_l, c_l)

                # --- verify the speculation against the AllReduce result ---
                s_g, inv_g, c_g = _emit_scale_chain(
                    nc, consts, gmax_g, gamma_sb, mask_t, expc_t, "g")
                dif = consts.tile([P, 1], I32)
                nc.vector.tensor_tensor(
                    out=dif,
                    in0=s_l.bitcast(I32),
                    in1=s_g.bitcast(I32),
                    op=mybir.AluOpType.bitwise_xor,
                )
                regs = nc.alloc_registers(
                    "spec_chk",
                    bass.OrderedSet([
                        mybir.EngineType.SP,
                        mybir.EngineType.DVE,
                        mybir.EngineType.Activation,
                        mybir.EngineType.PE,
                    ]),
                )
                for reg in regs:
                    nc.reg_load(reg, dif[0:1, 0:1])
                with tc.If(nc.snap(regs) != 0):
                    # mismatch: redo everything with the global scale
                    _emit_phase2(nc, pools, rows, xt_cols, y_rows, w_sb,
                                 bias_sb, negc_t, inv_g, c_g)

    nc.compile()
    return nc


def quantize_params(weight: np.ndarray, bias: np.ndarray):
    """Ternary-quantize weight/bias exactly as the reference (f64 math whose
    f32 rounding matches jax-f32; verified margins are orders of magnitude
    above f32 accumulation differences)."""
    w64 = weight.astype(np.float64)
    g_w = np.float32(np.abs(w64).mean())
    wi = np.clip(np.round(w64 / (np.float64(g_w) + EPS)), -1.0, 1.0)
    b64 = bias.astype(np.float64)
    g_b = np.float32(np.abs(b64).mean())
    bi = np.clip(np.round(b64 / (np.float64(g_b) + EPS)), -1.0, 1.0)
    bq = (bi * np.float64(g_b)).astype(np.float32)  # exact: {-g_b, 0, g_b}
    return wi, g_w, bq


_PROGRAM_CACHE: dict[int, bacc.Bacc] = {}


def _get_program(rows: int) -> bacc.Bacc:
    if rows not in _PROGRAM_CACHE:
        _PROGRAM_CACHE[rows] = build_program(rows)
    return _PROGRAM_CACHE[rows]


def prepare_in_maps(x: np.ndarray, weight: np.ndarray, bias: np.ndarray):
    x = np.asarray(x, dtype=np.float32)
    weight = np.asarray(weight, dtype=np.float32)
    bias = np.asarray(bias, dtype=np.float32)
    batch, rows, in_f = x.shape
    assert batch == N_CORES and in_f == IN_F and weight.shape == (OUT_F, IN_F)

    wi, g_w, bq = quantize_params(weight, bias)
    wq_t = np.ascontiguousarray(wi.T).astype(ml_dtypes.bfloat16)  # [in, out]
    bq_row = np.ascontiguousarray(bq.reshape(1, OUT_F))
    gq = np.array([[g_w]], dtype=np.float32)

    in_maps = []
    for c in range(N_CORES):
        in_maps.append(
            {
                "xt": np.ascontiguousarray(x[c].T),
                "wq": wq_t,
                "bq": bq_row,
                "gq": gq,
            }
        )
    return in_maps, rows


def kernel(x: np.ndarray, weight: np.ndarray, bias: np.ndarray) -> np.ndarray:
    in_maps, rows = prepare_in_maps(x, weight, bias)
    nc = _get_program(rows)
    res = bass_utils.run_bass_kernel_spmd(nc, in_maps, core_ids=list(range(N_CORES)))
    return np.stack([res.results[c]["y"] for c in range(N_CORES)], axis=0)


# revision 23
# speedup vs baseline: 1.2158x; 1.0627x over previous
"""BitNetLinear Trainium2 kernel (8 NeuronCores, SPMD data-parallel).

y = round(clip(x, +-127*s)/s)*s @ (ternary(W))^T + ternary(b)
with s = exp2(floor(log2(max|x|/127 + eps))) a power of two (global over x).

Sharding: batch dim (8) -> one batch of [4096, 1024] per core.
Host prep: x shard transposed to [in, rows] (PE contracts over partitions);
weight/bias ternary-quantized on host (reference does this in __init__);
ternary weight shipped as bf16 [in, out].

Device: phase 1 streams x computing the local absmax. Because
floor(log2(.)) commutes with max, the global power-of-two scale equals the
max of the per-core local scales, so each core SPECULATES with its local
scale and starts quantize+matmul immediately while a 512B AllReduce(max)
flies concurrently on the collectives hardware. At the end each core
compares its speculative scale against the global one and, on mismatch,
re-runs the (exact) quantize+matmul with the global scale under a
conditional branch. x is quantized to integer-valued bf16 (round-half-even
via the +-1.5*2^23 trick); the bf16 matmul with fp32 PSUM accumulation is
exact integer arithmetic (|x_int| <= 127, w in {-1,0,1}, |acc| < 2^24);
the result is scaled by s*gamma_w and the ternary bias is added.
"""

import numpy as np
import ml_dtypes
from contextlib import ExitStack

import concourse.bass as bass
import concourse.mybir as mybir
import concourse.tile as tile
from concourse import bacc, bass_isa, bass_utils

F32 = mybir.dt.float32
BF16 = mybir.dt.bfloat16
I32 = mybir.dt.int32

N_CORES = 8
P = 128
IN_F = 1024
OUT_F = 1024
KC = IN_F // P          # 8 contraction chunks
RSUB = 256              # rows quantized per chunk
ROUND_C = 12582912.0    # 1.5 * 2**23: float32 round-half-even trick
EPS = 1e-8


def _emit_scale_chain(nc, consts, gmax, gamma_sb, mask_t, expc_t, tag,
                      eng=None):
    """From a [P,1] absmax tile, compute s = exp2(floor(log2(m/127+eps)))
    via exponent masking, 1/s via exponent arithmetic, and c = s*gamma."""
    if eng is None:
        eng = nc.vector
    v_t = consts.tile([P, 1], F32, tag=f"v_{tag}")
    eng.tensor_scalar(
        out=v_t,
        in0=gmax,
        scalar1=float(np.float32(1.0 / 127.0)),
        scalar2=float(np.float32(EPS)),
        op0=mybir.AluOpType.mult,
        op1=mybir.AluOpType.add,
    )
    s_t = consts.tile([P, 1], F32, tag=f"s_{tag}")
    eng.tensor_tensor(
        out=s_t.bitcast(I32),
        in0=v_t.bitcast(I32),
        in1=mask_t,
        op=mybir.AluOpType.bitwise_and,
    )
    inv_t = consts.tile([P, 1], F32, tag=f"inv_{tag}")
    eng.tensor_tensor(
        out=inv_t.bitcast(I32),
        in0=expc_t,
        in1=s_t.bitcast(I32),
        op=mybir.AluOpType.subtract,
    )
    c_t = consts.tile([P, 1], F32, tag=f"c_{tag}")
    eng.tensor_mul(out=c_t, in0=s_t, in1=gamma_sb)
    return s_t, inv_t, c_t


def _emit_phase2(nc, pools, rows, xt_cols, y_rows, w_sb, bias_sb, negc_t,
                 inv_t, c_t):
    """Quantize x with 1/s, matmul against the ternary weight, scale by c,
    add bias, store y."""
    xq_pool, tq_pool, ub_pool, xi_pool, yo_pool, ps_pool = pools
    nhalf = OUT_F // 512
    for t in range(rows // RSUB):
        xc = xq_pool.tile([P, KC, RSUB], F32, tag="xc")
        nc.sync.dma_start(out=xc, in_=xt_cols[t])
        tq = tq_pool.tile([P, KC, RSUB], F32, tag="tq")
        nc.vector.tensor_scalar(
            out=tq,
            in0=xc,
            scalar1=inv_t,
            scalar2=ROUND_C,
            op0=mybir.AluOpType.mult,
            op1=mybir.AluOpType.add,
        )
        ub = ub_pool.tile([P, KC, RSUB], BF16, tag="ub")
        nc.scalar.activation(
            out=ub,
            in_=tq,
            func=mybir.ActivationFunctionType.Identity,
            bias=negc_t,
            scale=1.0,
        )
        xi = xi_pool.tile([P, KC, RSUB], BF16, tag="xi")
        nc.vector.tensor_scalar(
            out=xi,
            in0=ub,
            scalar1=-127.0,
            scalar2=127.0,
            op0=mybir.AluOpType.max,
            op1=mybir.AluOpType.min,
        )
        for h in range(RSUB // P):
            ps = ps_pool.tile([P, OUT_F], F32, tag="ps")
            for k in range(KC):
                for n in range(nhalf):
                    nc.tensor.matmul(
                        ps[:, n * 512 : (n + 1) * 512],
                        lhsT=xi[:, k, h * P : (h + 1) * P],
                        rhs=w_sb[:, k, n * 512 : (n + 1) * 512],
                        start=(k == 0),
                        stop=(k == KC - 1),
                    )
            yo = yo_pool.tile([P, OUT_F], F32, tag="yo")
            nc.scalar.activation(
                out=yo,
                in_=ps,
                func=mybir.ActivationFunctionType.Copy,
                bias=0.0,
                scale=c_t,
            )
            nc.vector.tensor_add(out=yo, in0=yo, in1=bias_sb)
            nc.sync.dma_start(out=y_rows[t * (RSUB // P) + h], in_=yo)


def build_program(rows: int = 4096, num_cores: int = N_CORES,
                  speculate: bool = True) -> bacc.Bacc:
    assert rows % RSUB == 0
    nc = bacc.Bacc(
        "TRN2",
        target_bir_lowering=False,
        debug=False,
        enable_asserts=False,
        num_devices=num_cores,
    )
    nt = rows // RSUB
    # x shard pre-tiled on host: xt[t, p, c, r] = x[t*RSUB + r, c*P + p],
    # so every phase-2 chunk load is one fully-contiguous 512 KiB read.
    xt = nc.dram_tensor("xt", (nt, P, KC, RSUB), F32, kind="ExternalInput").ap()
    wq = nc.dram_tensor("wq", (IN_F, OUT_F), BF16, kind="ExternalInput").ap()
    bq = nc.dram_tensor("bq", (1, OUT_F), F32, kind="ExternalInput").ap()
    gq = nc.dram_tensor("gq", (1, 1), F32, kind="ExternalInput").ap()
    y = nc.dram_tensor("y", (rows, OUT_F), F32, kind="ExternalOutput").ap()
    # Collectives cannot target I/O tensors; bounce through internal DRAM.
    cc_in = nc.dram_tensor("cc_in", (P, 1), F32).ap()
    cc_out = nc.dram_tensor("cc_out", (P, 1), F32).ap()

    with tile.TileContext(nc, num_cores=num_cores) as tc, ExitStack() as ctx:
        consts = ctx.enter_context(tc.tile_pool(name="consts", bufs=1))

        mask_t = consts.tile([P, 1], I32)
        nc.vector.memset(mask_t, -8388608)  # 0xFF800000: sign+exponent mask
        expc_t = consts.tile([P, 1], I32)
        nc.vector.memset(expc_t, 0x7F000000)  # bits of (254<<23)
        negc_t = consts.tile([P, 1], F32)
        nc.vector.memset(negc_t, -ROUND_C)

        # --- phase 1: local absmax of the x shard (two t-chunks per DMA
        # for ~2 MiB transfers) ---
        g = 2 if nt % 2 == 0 else 1
        xt_pairs = xt.rearrange("(j g) p c r -> j p g c r", g=g)
        partials = consts.tile([P, nt // g], F32)
        with tc.tile_pool(name="xmax", bufs=4) as xpool:
            for j in range(nt // g):
                xsb = xpool.tile([P, g, KC, RSUB], F32)
                nc.sync.dma_start(out=xsb, in_=xt_pairs[j])
                nc.vector.tensor_reduce(
                    out=partials[:, j : j + 1],
                    in_=xsb,
                    axis=mybir.AxisListType.XYZ,
                    op=mybir.AluOpType.max,
                    apply_absolute_value=True,
                )
        lmax = consts.tile([P, 1], F32)
        nc.vector.tensor_reduce(
            out=lmax, in_=partials, axis=mybir.AxisListType.X,
            op=mybir.AluOpType.max,
        )
        gmax_l = consts.tile([P, 1], F32)
        nc.gpsimd.partition_all_reduce(
            gmax_l, lmax, channels=P, reduce_op=bass_isa.ReduceOp.max
        )

        # --- global max across the 8 cores (gpsimd queue so the bounce
        # DMAs never block the sync queue that feeds phase 2) ---
        nc.gpsimd.dma_start(out=cc_in, in_=gmax_l)
        nc.gpsimd.collective_compute(
            "AllReduce",
            mybir.AluOpType.max,
            replica_groups=[list(range(num_cores))],
            ins=[cc_in.opt()],
            outs=[cc_out.opt()],
        )
        gmax_g = consts.tile([P, 1], F32)
        nc.gpsimd.dma_start(out=gmax_g, in_=cc_out)

        # --- constants: ternary weight [p, kc, o], bias row, gamma scalar ---
        w_sb = consts.tile([P, KC, OUT_F], BF16)
        nc.sync.dma_start(out=w_sb, in_=wq.rearrange("(c p) o -> p c o", p=P))
        bias_sb = consts.tile([P, OUT_F], F32)
        nc.sync.dma_start(out=bias_sb, in_=bq.to_broadcast((P, OUT_F)))
        gamma_sb = consts.tile([P, 1], F32)
        nc.sync.dma_start(out=gamma_sb, in_=gq.to_broadcast((P, 1)))

        # --- PE warmup: junk matmuls gated on the local max; they flip HAM
        # to full clock while the scale chain + first quantize run ---
        warm_rhs = consts.tile([P, 512], BF16)
        nc.vector.memset(warm_rhs, 0.0)
        nc.vector.tensor_copy(out=warm_rhs[:, 0:1], in_=gmax_l)
        with tc.tile_pool(name="warm_ps", bufs=1, space="PSUM") as warm_pool:
            warm_ps = warm_pool.tile([P, 512], F32)
            for _ in range(16):
                nc.tensor.matmul(
                    warm_ps, lhsT=w_sb[:, 0, 0:P], rhs=warm_rhs,
                    start=True, stop=True,
                )

        xt_cols = xt
        y_rows = y.rearrange("(t p) o -> t p o", p=P)

        with (
            tc.tile_pool(name="xq", bufs=6) as xq_pool,
            tc.tile_pool(name="tq", bufs=3) as tq_pool,
            tc.tile_pool(name="ub", bufs=3) as ub_pool,
            tc.tile_pool(name="xi", bufs=4) as xi_pool,
            tc.tile_pool(name="yo", bufs=4) as yo_pool,
            tc.tile_pool(name="ps", bufs=4, space="PSUM") as ps_pool,
        ):
            pools = (xq_pool, tq_pool, ub_pool, xi_pool, yo_pool, ps_pool)
            if not speculate:
                _, inv_g, c_g = _emit_scale_chain(
                    nc, consts, gmax_g, gamma_sb, mask_t, expc_t, "g")
                _emit_phase2(nc, pools, rows, xt_cols, y_rows, w_sb, bias_sb,
                             negc_t, inv_g, c_g)
            else:
                s_l, inv_l, c_l = _emit_scale_chain(
                    nc, consts, gmax_l, gamma_sb, mask_t, expc_t, "l")
                _emit_phase2(nc, pools, rows, xt_cols, y_rows, w_sb, bias_sb,
                             negc_t, inv_l, c_l)

                # --- verify the speculation against the AllReduce result ---
                s_g, inv_g, c_g = _emit_scale_chain(
                    nc, consts, gmax_g, gamma_sb, mask_t, expc_t, "g")
                dif = consts.tile([P, 1], I32)
                nc.vector.tensor_tensor(
                    out=dif,
                    in0=s_l.bitcast(I32),
                    in1=s_g.bitcast(I32),
                    op=mybir.AluOpType.bitwise_xor,
                )
                regs = nc.alloc_registers(
                    "spec_chk",
                    bass.OrderedSet([
                        mybir.EngineType.SP,
                        mybir.EngineType.DVE,
                        mybir.EngineType.Activation,
                        mybir.EngineType.PE,
                    ]),
                )
                for reg in regs:
                    nc.reg_load(reg, dif[0:1, 0:1])
                with tc.If(nc.snap(regs) != 0):
                    # mismatch: redo everything with the global scale
                    _emit_phase2(nc, pools, rows, xt_cols, y_rows, w_sb,
                                 bias_sb, negc_t, inv_g, c_g)

    nc.compile()
    return nc


def quantize_params(weight: np.ndarray, bias: np.ndarray):
    """Ternary-quantize weight/bias exactly as the reference (f64 math whose
    f32 rounding matches jax-f32; verified margins are orders of magnitude
    above f32 accumulation differences)."""
    w64 = weight.astype(np.float64)
    g_w = np.float32(np.abs(w64).mean())
    wi = np.clip(np.round(w64 / (np.float64(g_w) + EPS)), -1.0, 1.0)
    b64 = bias.astype(np.float64)
    g_b = np.float32(np.abs(b64).mean())
    bi = np.clip(np.round(b64 / (np.float64(g_b) + EPS)), -1.0, 1.0)
    bq = (bi * np.float64(g_b)).astype(np.float32)  # exact: {-g_b, 0, g_b}
    return wi, g_w, bq


_PROGRAM_CACHE: dict[int, bacc.Bacc] = {}


def _get_program(rows: int) -> bacc.Bacc:
    if rows not in _PROGRAM_CACHE:
        _PROGRAM_CACHE[rows] = build_program(rows)
    return _PROGRAM_CACHE[rows]


def tile_x_shard(x2d: np.ndarray) -> np.ndarray:
    """[rows, IN_F] -> [nt, P, KC, RSUB] with xt[t,p,c,r] = x[t*RSUB+r, c*P+p]."""
    rows = x2d.shape[0]
    return np.ascontiguousarray(
        x2d.reshape(rows // RSUB, RSUB, KC, P).transpose(0, 3, 2, 1)
    )


def prepare_in_maps(x: np.ndarray, weight: np.ndarray, bias: np.ndarray):
    x = np.asarray(x, dtype=np.float32)
    weight = np.asarray(weight, dtype=np.float32)
    bias = np.asarray(bias, dtype=np.float32)
    batch, rows, in_f = x.shape
    assert batch == N_CORES and in_f == IN_F and weight.shape == (OUT_F, IN_F)

    wi, g_w, bq = quantize_params(weight, bias)
    wq_t = np.ascontiguousarray(wi.T).astype(ml_dtypes.bfloat16)  # [in, out]
    bq_row = np.ascontiguousarray(bq.reshape(1, OUT_F))
    gq = np.array([[g_w]], dtype=np.float32)

    in_maps = []
    for c in range(N_CORES):
        in_maps.append(
            {
                "xt": tile_x_shard(x[c]),
                "wq": wq_t,
                "bq": bq_row,
                "gq": gq,
            }
        )
    return in_maps, rows


def kernel(x: np.ndarray, weight: np.ndarray, bias: np.ndarray) -> np.ndarray:
    in_maps, rows = prepare_in_maps(x, weight, bias)
    nc = _get_program(rows)
    res = bass_utils.run_bass_kernel_spmd(nc, in_maps, core_ids=list(range(N_CORES)))
    return np.stack([res.results[c]["y"] for c in range(N_CORES)], axis=0)
